# revision 15
# baseline (speedup 1.0000x reference)
"""Causal single-head attention on 8 Trainium2 NeuronCores (Bass/Tile).

Problem: X[4,4096,512] fp32, Wq/Wk/Wv[512,64] fp32.
  Q=XWq, K=XWk, V=XWv ; Z = softmax(mask(QK^T)/8) V    -> [4,4096,64]

Sharding: 2 cores per batch, fully uniform SPMD program.
  - Keys/values are split by PARITY of 128-row key blocks: core A of a pair
    owns even key blocks, core B odd ones.  Each core's X^T input is
    ROTATED left by 128*parity columns by the host, which makes "my key
    blocks" sit at even 128-col positions for BOTH cores -- so one
    instruction stream with static addresses serves both.
  - Each core computes, for every query tile, partial attention over its
    own half of the keys with un-normalized softmax (no max subtraction --
    logits here are ~N(0, 0.2^2) so exp cannot overflow):
        numerator   N_c = sum_k exp(s)*V,   denominator D_c = sum_k exp(s)
    The host combines  Z = (N_A + N_B) / (D_A + D_B)  exactly.  The
    rotation wraps one query block on core B (tile 7); the host simply
    uses A-only partials for those 128 queries (A covers them fully).
  - Denominators come for free as column 64 of V_ext = [V | 1] in the
    P^T @ V_ext matmul.

v2 speedups over the bf16 baseline (65.9us):
  - Early HAM warmup: memset-fed garbage matmuls run from the moment the
    engines clear the start barrier, so the PE clock-gate (1.2 -> 2.4 GHz)
    is released before the first real matmul instead of ~6us into them.
  - PV matmuls for tiles t>=1 run in fp8e4 DoubleRow mode: one matmul per
    PAIR of key blocks (virtual contraction 256), ~2x fewer PE cycles.
    P (=exp scores) is written by ACT directly as fp8; V is evacuated to
    an fp8 [V|1] buffer with 80-col block stride (16B-aligned for the DR
    weight AP).  Tile 0 (queries 0..511, the precision-critical ones with
    little averaging) keeps the full-bf16 path; numpy simulation shows the
    end-to-end max-rel-err stays at the bf16 baseline's 3.5e-3.
  - The exp for the DIAGONAL group of tiles t>=2 moves off the
    (bottleneck) ACT engine onto DVE via the classic exp2 bit-trick:
    i16 = round(s * log2e*128*scale + (127*128 - C)) bitcast to bf16 is
    exp(s*scale) to ~3%; a fused scalar_tensor_tensor applies the causal
    mask and converts to fp8 in the same op.  ~3% error on those blocks is
    invisible after averaging (verified in numpy).
  - Causal masking is applied only where the mask is actually partial:
    a 128-col triangular window per diagonal block (the rest of the old
    512/256-wide mask multiplies were by 1.0).
"""

import numpy as np
import ml_dtypes

import concourse.bacc as bacc
import concourse.bass as bass
import concourse.mybir as mybir
import concourse.tile as tile

B, S, DIN, E = 4, 4096, 512, 64
PB = 128            # partition / key block
QT = 512            # query tile width
NQT = S // QT       # 8 query tiles
NKB = S // PB       # 32 key blocks per batch
HKB = NKB // 2      # 16 packed key blocks per core
SH = S // 2         # 2048 packed keys per core
NCORES = 8
SCALE = 1.0 / np.sqrt(E)
GJ = 2              # k-blocks per exp group (PSUM banks = GJ)
VS = 80             # fp8 V_ext block stride (65 used; 80 keeps 16B align)
BT_TILES = frozenset({2, 3, 4, 5, 6})   # DVE bit-trick exp on diag groups
N_WARM = 7          # early HAM-warmup matmuls (512-col)

# bit-trick constants: exp(s*SCALE) ~ bitcast_bf16(i16(round(s*A + BIAS)))
BT_A = float(np.log2(np.e) * 128.0 * SCALE)
BT_BIAS = float(127.0 * 128.0 - 0.0430 * 128.0)

BF16 = ml_dtypes.bfloat16
F8E4 = ml_dtypes.float8_e4m3
BF = mybir.dt.bfloat16
F32 = mybir.dt.float32
F8 = mybir.dt.float8e4
I16 = mybir.dt.int16
DR = mybir.MatmulPerfMode.DoubleRow

_CACHE = {}


def _build():
    nc = bacc.Bacc("TRN2", target_bir_lowering=False, debug=False,
                   enable_asserts=False, num_devices=NCORES)

    xtf_h = nc.dram_tensor("xtf", [DIN, S], BF, kind="ExternalInput")
    wq2_h = nc.dram_tensor("wq2", [DIN, 2 * E], BF, kind="ExternalInput")
    wk2_h = nc.dram_tensor("wk2", [DIN, 2 * E], BF, kind="ExternalInput")
    wv1_h = nc.dram_tensor("wv1", [DIN, E], BF, kind="ExternalInput")
    # [tri|tri] 2x128 windows + [128,768] full mask for the bit-trick path
    msk_h = nc.dram_tensor("msk", [PB, 256 + 768], BF, kind="ExternalInput")
    zt_h = nc.dram_tensor("zt", [E + 1, S], F32, kind="ExternalOutput")

    xtf_r = xtf_h.ap().rearrange("(c p) s -> p c s", p=PB)
    zt = zt_h.ap()

    with tile.TileContext(nc) as tc:
        with (
            tc.tile_pool(name="big", bufs=1) as big,
            tc.tile_pool(name="pt", bufs=8) as ptp,
            tc.tile_pool(name="it", bufs=2) as itp,
            tc.tile_pool(name="zsb", bufs=2) as zsbp,
            tc.tile_pool(name="ppsum", bufs=3, space="PSUM") as pp,
            tc.tile_pool(name="spsum", bufs=2, space="PSUM") as sp,
            tc.tile_pool(name="zpsum", bufs=1, space="PSUM") as zp,
        ):
            # ---- persistent SBUF buffers ----
            xtf_sb = big.tile([PB, 4, S], BF, tag="xtf")
            wq2_sb = big.tile([PB, 4, 2 * E], BF, tag="wq2")
            wk2_sb = big.tile([PB, 4, 2 * E], BF, tag="wk2")
            wv1_sb = big.tile([PB, 4, E], BF, tag="wv1")
            msk_sb = big.tile([PB, 256 + 768], BF, tag="msk")
            qt2 = big.tile([PB, S], BF, tag="qt2")      # doubled Q^T (rot)
            kt2 = big.tile([PB, SH], BF, tag="kt2")     # doubled K^T (packed)
            vext8 = big.tile([PB, HKB, VS], F8, tag="vext8")   # fp8 [V|1|pad]
            vext16 = big.tile([PB, 2, E + 1], BF, tag="vext16")  # t=0 blocks
            warm_sb = big.tile([PB, QT], BF, tag="warm")

            dma = nc.sync.dma_start

            # HAM warmup: memset-fed garbage matmuls keep the PE busy from
            # the moment the start barrier clears, so the clock gate is at
            # 8/8 before the first real projection chain (DMA-gated) runs.
            nc.vector.memset(warm_sb[:], 0.25)
            warm_ps = pp.tile([PB, QT], F32, tag="proj", name="warm_ps")
            for _ in range(N_WARM):
                nc.tensor.matmul(warm_ps[:], warm_sb[:, 0:PB], warm_sb[:],
                                 start=True, stop=True)

            # ---- input DMAs, ordered by first consumption.  Weights first
            # (small, land fast); X^T's first kilocol in two 512-col pieces
            # so tile 0's Q and half-K chains start ~4us earlier.
            dma(xtf_sb[:, 0, 0:QT], xtf_r[:, 0, 0:QT])
            dma(wq2_sb[:], wq2_h.ap().rearrange("(c p) m -> p c m", p=PB))
            dma(wk2_sb[:], wk2_h.ap().rearrange("(c p) m -> p c m", p=PB))
            for c in range(1, 4):      # rest of X^T cols 0:512, per chunk
                dma(xtf_sb[:, c, 0:QT], xtf_r[:, c, 0:QT])
            dma(xtf_sb[:, :, QT:2 * QT], xtf_r[:, :, QT:2 * QT])
            dma(wv1_sb[:], wv1_h.ap().rearrange("(c p) m -> p c m", p=PB))
            dma(xtf_sb[:, :, 2 * QT:3 * QT], xtf_r[:, :, 2 * QT:3 * QT])
            dma(msk_sb[:], msk_h.ap())
            for lo, hi in ((3 * QT, 5 * QT), (5 * QT, 7 * QT), (7 * QT, S)):
                dma(xtf_sb[:, :, lo:hi], xtf_r[:, :, lo:hi])

            # ones columns of V_ext (V blocks overwrite cols 0..63 later)
            nc.vector.memset(vext8[:, :, E:E + 1], 1.0)
            nc.vector.memset(vext16[:, :, E:E + 1], 1.0)

            msk_tri = msk_sb[:, 0:256].rearrange("p (a b) -> p a b", a=2)
            msk768 = msk_sb[:, 256:256 + 768]

            def even_blocks(ap2d, s4):
                """[128, 512] strided view: even 128-col blocks
                {8s4, 8s4+2, 8s4+4, 8s4+6} of a [128, S] AP."""
                seg = ap2d[:, 1024 * s4:1024 * (s4 + 1)]
                return seg.rearrange("p (b two x) -> p b two x",
                                     two=2, x=PB)[:, :, 0, :]

            def even_blocks_half(ap2d, h8):
                """[128, 256] strided view: even 128-col blocks of a
                512-col segment (half of an even_blocks segment)."""
                seg = ap2d[:, QT * h8:QT * (h8 + 1)]
                return seg.rearrange("p (b two x) -> p b two x",
                                     two=2, x=PB)[:, :, 0, :]

            # Projection chains: specs is a list of ('q', t) | ('k', s4)
            # | ('v', j).  Chains are interleaved per weight chunk so
            # consecutive matmuls alternate PSUM banks (hides PE drain) and
            # short V matmuls ride inside long 512-col streams (their
            # weight loads hide under the 512-col matmuls).
            def chains(*specs):
                tiles = [pp.tile([PB, QT], F32, tag="proj",
                                 name=f"{kind}_ps")
                         for kind, idx in specs]
                for c in range(4):
                    for (kind, idx), ps in zip(specs, tiles):
                        if kind == 'q':
                            nc.tensor.matmul(
                                ps[:], wq2_sb[:, c, :],
                                xtf_sb[:, c, QT * idx:QT * (idx + 1)],
                                start=(c == 0), stop=(c == 3))
                        elif kind == 'k':
                            nc.tensor.matmul(
                                ps[:], wk2_sb[:, c, :],
                                even_blocks(xtf_sb[:, c, :], idx),
                                start=(c == 0), stop=(c == 3))
                        elif kind == 'kh':
                            nc.tensor.matmul(
                                ps[:, 0:256], wk2_sb[:, c, :],
                                even_blocks_half(xtf_sb[:, c, :], idx),
                                start=(c == 0), stop=(c == 3))
                        else:
                            nc.tensor.matmul(
                                ps[:, 0:E],
                                xtf_sb[:, c, 2 * PB * idx:2 * PB * idx + PB],
                                wv1_sb[:, c, :],
                                start=(c == 0), stop=(c == 3))
                for (kind, idx), ps in zip(specs, tiles):
                    if kind == 'q':
                        nc.vector.tensor_copy(
                            qt2[:, QT * idx:QT * (idx + 1)], ps[:])
                    elif kind == 'k':
                        nc.vector.tensor_copy(
                            kt2[:, QT * idx:QT * (idx + 1)], ps[:])
                    elif kind == 'kh':
                        nc.vector.tensor_copy(
                            kt2[:, 256 * idx:256 * (idx + 1)], ps[:, 0:256])
                    else:
                        nc.vector.tensor_copy(
                            vext8[:, idx, 0:E], ps[:, 0:E])
                        if idx < 2:
                            nc.vector.tensor_copy(
                                vext16[:, idx, 0:E], ps[:, 0:E])

            # ---- main loop over query tiles ----
            pend = []       # deferred PV groups (keeps PE off ACT's tail)
            for t in range(NQT):
                # V blocks (2t, 2t+1) are only read by the deferred PV of
                # tile t, flushed during t+1 -- project them one tile late,
                # riding inside that tile's long chains.
                if t == 0:
                    chains(('q', 0))
                    chains(('kh', 0))        # kt2 cols 0:256 from X cols 0:512
                elif t == 1:
                    chains(('q', 1), ('q', 2), ('v', 0))
                    chains(('kh', 1), ('v', 1))   # kt2 cols 256:512
                elif t == 7:
                    chains(('q', 7), ('v', 12), ('v', 13))
                    chains(('v', 14), ('v', 15))
                elif t % 2 == 1:
                    chains(('q', t), ('q', t + 1), ('v', 2 * t - 2))
                    chains(('v', 2 * t - 1))
                else:
                    chains(('k', t // 2), ('v', 2 * t - 2), ('v', 2 * t - 1))

                z_ps = zp.tile([E + 1, QT], F32, tag="z", name="z_ps")
                njb = 2 * t + 2
                groups = [list(range(g, min(g + GJ, njb)))
                          for g in range(0, njb, GJ)]
                for js in groups:
                    s_ps = sp.tile([PB, GJ * QT], F32, tag="s", name="s_ps")
                    for j in js:
                        sl = j - js[0]
                        half = slice(0, 64) if j % 2 == 0 else slice(64, 128)
                        if j == 2 * t + 1:
                            # diagonal-odd block: cols [0,256) fully masked,
                            # compute only the live half
                            nc.tensor.matmul(
                                s_ps[:, QT * sl:QT * sl + 256],
                                kt2[half, PB * j:PB * (j + 1)],
                                qt2[half, QT * t + 256:QT * (t + 1)],
                                start=True, stop=True)
                        else:
                            nc.tensor.matmul(
                                s_ps[:, QT * sl:QT * (sl + 1)],
                                kt2[half, PB * j:PB * (j + 1)],
                                qt2[half, QT * t:QT * (t + 1)],
                                start=True, stop=True)

                    # flush deferred PV matmuls (keep a few in flight;
                    # drain harder on the last tile to shorten the tail)
                    lim = 6 if t < 7 else 2
                    if len(pend) >= lim:
                        _flush_pv(nc, pend.pop(0))

                    diag = js[-1] == 2 * t + 1
                    w = QT * len(js)
                    if diag:
                        w -= 256     # diagonal-odd block is half width
                    if t == 0:
                        pt = ptp.tile([PB, GJ * QT], BF, tag="pt0",
                                      name="pt0")
                    else:
                        pt = ptp.tile([PB, GJ * QT], F8, tag="pt", name="pt")
                    bt_offdiag = (not diag) and t == 7 and js[0] in (6, 10)
                    if (diag and t in BT_TILES) or bt_offdiag:
                        # DVE bit-trick exp (+ fused mask on diag groups)
                        it16 = itp.tile([PB, GJ * QT], I16, tag="it",
                                        name="it16")
                        nc.vector.tensor_scalar(
                            it16[:, 0:w], s_ps[:, 0:w], BT_A, BT_BIAS,
                            mybir.AluOpType.mult, mybir.AluOpType.add)
                        if diag:
                            nc.vector.scalar_tensor_tensor(
                                pt[:, 0:w], it16[:, 0:w].bitcast(BF), 1.0,
                                msk768[:, 0:w],
                                mybir.AluOpType.mult, mybir.AluOpType.mult)
                        else:
                            nc.vector.tensor_copy(
                                pt[:, 0:w], it16[:, 0:w].bitcast(BF))
                    else:
                        nc.scalar.activation(pt[:, 0:w], s_ps[:, 0:w],
                                             mybir.ActivationFunctionType.Exp,
                                             scale=float(SCALE))
                        if diag:
                            # partial-mask windows: first 128 cols of the
                            # even block and of the odd live half
                            pt_win = pt[:, 0:QT + PB].rearrange(
                                "p (a b) -> p a b", b=PB)[:, 0:5:4, :]
                            nc.vector.tensor_mul(pt_win, pt_win, msk_tri)
                    pend.append((z_ps, vext8, vext16, pt, js, t))

                # attach Z evacuation of this tile to the last deferred group
                pend[-1] = pend[-1] + (zt, zsbp)

            # tail: flush remaining deferred groups
            for p in pend:
                _flush_pv(nc, p)

    nc.compile()
    return nc


def _flush_pv(nc, pend):
    """Emit the deferred PV matmul group (and Z evacuation if attached)."""
    z_ps, vext8, vext16, pt, js, t = pend[:6]
    a = js[0]
    if t == 0:
        # bf16 path (precision-critical first queries)
        nc.tensor.matmul(z_ps[:], vext16[:, 0, :], pt[:, 0:QT],
                         start=True, stop=False)
        nc.tensor.matmul(z_ps[:, 256:QT], vext16[:, 1, :],
                         pt[:, QT:QT + 256], start=False, stop=True)
    elif js[-1] == 2 * t + 1:
        # diagonal group: plain fp8 matmul for the even block's first 256
        # queries, DoubleRow for the shared last 256 queries
        nc.tensor.matmul(z_ps[:, 0:256], vext8[:, a, 0:E + 1],
                         pt[:, 0:256], start=False, stop=False)
        nc.tensor.matmul(
            z_ps[:, 256:QT], vext8[:, a:a + 2, 0:E + 1],
            pt[:, 256:QT + 256].rearrange("p (k q) -> p k q", k=2),
            start=False, stop=True, perf_mode=DR)
    else:
        # off-diagonal pair: one DoubleRow matmul (virtual contraction 256)
        nc.tensor.matmul(
            z_ps[:], vext8[:, a:a + 2, 0:E + 1],
            pt[:].rearrange("p (k q) -> p k q", k=2),
            start=(a == 0), stop=False, perf_mode=DR)
    if len(pend) > 6:
        zt, zsbp = pend[6], pend[7]
        z_sb = zsbp.tile([E + 1, QT], F32, tag="zsb", name="z_sb")
        nc.vector.tensor_copy(z_sb[:], z_ps[:])
        nc.sync.dma_start(zt[:, QT * t:QT * (t + 1)], z_sb[:])


def _get_nc():
    if "nc" not in _CACHE:
        _CACHE["nc"] = _build()
    return _CACHE["nc"]


def _host_inputs(X, Wq, Wk, Wv):
    """Per-core input maps. Core 2b+c: batch b, key parity c; X^T rotated
    left by 128*c columns."""
    w2 = lambda w: np.concatenate([w, w], axis=1).astype(BF16)
    wq2, wk2 = w2(Wq), w2(Wk)
    wv1 = Wv.astype(BF16)
    # masks: [tri|tri] (2x128 partial windows) + 768-wide full diag mask
    i = np.arange(PB)[:, None]
    tri = (i <= np.arange(PB)[None, :]).astype(BF16)
    v512 = np.arange(QT)[None, :]
    even_m = (i <= v512).astype(BF16)            # [128, 512]
    v256 = np.arange(256)[None, :]
    odd_m = (i <= v256).astype(BF16)             # [128, 256]
    msk = np.concatenate([tri, tri, even_m, odd_m], axis=1)  # [128, 1024]

    in_maps = []
    for b in range(B):
        xt = np.ascontiguousarray(np.asarray(X[b]).T).astype(BF16)
        for c in (0, 1):
            xtc = xt if c == 0 else np.ascontiguousarray(
                np.roll(xt, -PB * c, axis=1))
            in_maps.append({
                "xtf": xtc,
                "wq2": wq2, "wk2": wk2, "wv1": wv1, "msk": msk,
            })
    return in_maps


def _combine(results):
    Z = np.empty((B, S, E), np.float32)
    for b in range(B):
        za = results[2 * b]["zt"].astype(np.float32)
        zb = np.roll(results[2 * b + 1]["zt"].astype(np.float32),
                     PB, axis=1)     # un-rotate core B's query columns
        # B's wrapped query block (global q < 128) is garbage; A covers it.
        zb[:, 0:PB] = 0.0
        num = za[:E] + zb[:E]
        den = za[E] + zb[E]
        Z[b] = (num / den[None, :]).T
    return Z


def kernel(X, Wq, Wk, Wv, _trace=False, _tmpdir=None):
    from concourse.bass_utils import run_bass_kernel_spmd
    nc = _get_nc()
    in_maps = _host_inputs(X, Wq, Wk, Wv)
    kw = {}
    if _tmpdir is not None:
        kw["tmpdir"] = _tmpdir
    res = run_bass_kernel_spmd(nc, in_maps, core_ids=list(range(NCORES)),
                               trace=_trace, **kw)
    _CACHE["last"] = res
    return _combine(res.results)


# revision 18
# speedup vs baseline: 1.0682x; 1.0682x over previous
"""Causal single-head attention on 8 Trainium2 NeuronCores (Bass/Tile).

Problem: X[4,4096,512] fp32, Wq/Wk/Wv[512,64] fp32.
  Q=XWq, K=XWk, V=XWv ; Z = softmax(mask(QK^T)/8) V    -> [4,4096,64]

Sharding: 2 cores per batch, fully uniform SPMD program.
  - Keys/values are split by PARITY of 128-row key blocks: core A of a pair
    owns even key blocks, core B odd ones.  Each core's X^T input is
    ROTATED left by 128*parity columns by the host, which makes "my key
    blocks" sit at even 128-col positions for BOTH cores -- so one
    instruction stream with static addresses serves both.
  - Each core computes, for every query tile, partial attention over its
    own half of the keys with un-normalized softmax (no max subtraction --
    logits here are ~N(0, 0.2^2) so exp cannot overflow):
        numerator   N_c = sum_k exp(s)*V,   denominator D_c = sum_k exp(s)
    The host combines  Z = (N_A + N_B) / (D_A + D_B)  exactly.  The
    rotation wraps one query block on core B (tile 7); the host simply
    uses A-only partials for those 128 queries (A covers them fully).
  - Denominators come for free as column 64 of V_ext = [V | 1] in the
    P^T @ V_ext matmul.

v2 speedups over the bf16 baseline (65.9us):
  - Early HAM warmup: memset-fed garbage matmuls run from the moment the
    engines clear the start barrier, so the PE clock-gate (1.2 -> 2.4 GHz)
    is released before the first real matmul instead of ~6us into them.
  - PV matmuls for tiles t>=1 run in fp8e4 DoubleRow mode: one matmul per
    PAIR of key blocks (virtual contraction 256), ~2x fewer PE cycles.
    P (=exp scores) is written by ACT directly as fp8; V is evacuated to
    an fp8 [V|1] buffer with 80-col block stride (16B-aligned for the DR
    weight AP).  Tile 0 (queries 0..511, the precision-critical ones with
    little averaging) keeps the full-bf16 path; numpy simulation shows the
    end-to-end max-rel-err stays at the bf16 baseline's 3.5e-3.
  - The exp for the DIAGONAL group of tiles t>=2 moves off the
    (bottleneck) ACT engine onto DVE via the classic exp2 bit-trick:
    i16 = round(s * log2e*128*scale + (127*128 - C)) bitcast to bf16 is
    exp(s*scale) to ~3%; a fused scalar_tensor_tensor applies the causal
    mask and converts to fp8 in the same op.  ~3% error on those blocks is
    invisible after averaging (verified in numpy).
  - Causal masking is applied only where the mask is actually partial:
    a 128-col triangular window per diagonal block (the rest of the old
    512/256-wide mask multiplies were by 1.0).
"""

import numpy as np
import ml_dtypes

import concourse.bacc as bacc
import concourse.bass as bass
import concourse.mybir as mybir
import concourse.tile as tile

B, S, DIN, E = 4, 4096, 512, 64
PB = 128            # partition / key block
QT = 512            # query tile width
NQT = S // QT       # 8 query tiles
NKB = S // PB       # 32 key blocks per batch
HKB = NKB // 2      # 16 packed key blocks per core
SH = S // 2         # 2048 packed keys per core
NCORES = 8
SCALE = 1.0 / np.sqrt(E)
GJ = 2              # k-blocks per exp group (PSUM banks = GJ)
VS = 80             # fp8 V_ext block stride (65 used; 80 keeps 16B align)
BT_TILES = frozenset({2, 3, 4, 5, 6})   # DVE bit-trick exp on diag groups
N_WARM = 9          # early HAM-warmup matmuls (512-col)

# bit-trick constants: exp(s*SCALE) ~ bitcast_bf16(i16(round(s*A + BIAS)))
BT_A = float(np.log2(np.e) * 128.0 * SCALE)
BT_BIAS = float(127.0 * 128.0 - 0.0430 * 128.0)

BF16 = ml_dtypes.bfloat16
F8E4 = ml_dtypes.float8_e4m3
BF = mybir.dt.bfloat16
F32 = mybir.dt.float32
F8 = mybir.dt.float8e4
I16 = mybir.dt.int16
DR = mybir.MatmulPerfMode.DoubleRow

_CACHE = {}


def _build():
    nc = bacc.Bacc("TRN2", target_bir_lowering=False, debug=False,
                   enable_asserts=False, num_devices=NCORES)

    xtf_h = nc.dram_tensor("xtf", [DIN, S], BF, kind="ExternalInput")
    wq2_h = nc.dram_tensor("wq2", [DIN, 2 * E], BF, kind="ExternalInput")
    wk2_h = nc.dram_tensor("wk2", [DIN, 2 * E], BF, kind="ExternalInput")
    wv1_h = nc.dram_tensor("wv1", [DIN, E], BF, kind="ExternalInput")
    # [tri|tri] 2x128 windows + [128,768] full mask for the bit-trick path
    msk_h = nc.dram_tensor("msk", [PB, 256 + 768], BF, kind="ExternalInput")
    zt_h = nc.dram_tensor("zt", [E + 1, S], F32, kind="ExternalOutput")

    xtf_r = xtf_h.ap().rearrange("(c p) s -> p c s", p=PB)
    zt = zt_h.ap()

    with tile.TileContext(nc) as tc:
        with (
            tc.tile_pool(name="big", bufs=1) as big,
            tc.tile_pool(name="pt", bufs=8) as ptp,
            tc.tile_pool(name="it", bufs=2) as itp,
            tc.tile_pool(name="zsb", bufs=2) as zsbp,
            tc.tile_pool(name="ppsum", bufs=3, space="PSUM") as pp,
            tc.tile_pool(name="spsum", bufs=2, space="PSUM") as sp,
            tc.tile_pool(name="zpsum", bufs=1, space="PSUM") as zp,
        ):
            # ---- persistent SBUF buffers ----
            xtf_sb = big.tile([PB, 4, S], BF, tag="xtf")
            wq2_sb = big.tile([PB, 4, 2 * E], BF, tag="wq2")
            wk2_sb = big.tile([PB, 4, 2 * E], BF, tag="wk2")
            wv1_sb = big.tile([PB, 4, E], BF, tag="wv1")
            msk_sb = big.tile([PB, 256 + 768], BF, tag="msk")
            qt2 = big.tile([PB, S], BF, tag="qt2")      # doubled Q^T (rot)
            kt2 = big.tile([PB, SH], BF, tag="kt2")     # doubled K^T (packed)
            vext8 = big.tile([PB, HKB, VS], F8, tag="vext8")   # fp8 [V|1|pad]
            vext16 = big.tile([PB, 2, E + 1], BF, tag="vext16")  # t=0 blocks
            warm_sb = big.tile([PB, QT], BF, tag="warm")

            dma = nc.sync.dma_start

            # HAM warmup: memset-fed garbage matmuls keep the PE busy from
            # the moment the start barrier clears, so the clock gate is at
            # 8/8 before the first real projection chain (DMA-gated) runs.
            nc.vector.memset(warm_sb[:], 0.25)
            warm_ps = pp.tile([PB, QT], F32, tag="proj", name="warm_ps")
            for _ in range(N_WARM):
                nc.tensor.matmul(warm_ps[:], warm_sb[:, 0:PB], warm_sb[:],
                                 start=True, stop=True)

            # ---- input DMAs, ordered by first consumption.  Weights first
            # (small, land fast); X^T's first kilocol in two 512-col pieces
            # so tile 0's Q and half-K chains start ~4us earlier.
            dma(xtf_sb[:, :, 0:QT], xtf_r[:, :, 0:QT])
            dma(wq2_sb[:], wq2_h.ap().rearrange("(c p) m -> p c m", p=PB))
            dma(wk2_sb[:], wk2_h.ap().rearrange("(c p) m -> p c m", p=PB))
            dma(xtf_sb[:, :, QT:2 * QT], xtf_r[:, :, QT:2 * QT])
            dma(wv1_sb[:], wv1_h.ap().rearrange("(c p) m -> p c m", p=PB))
            dma(xtf_sb[:, :, 2 * QT:3 * QT], xtf_r[:, :, 2 * QT:3 * QT])
            dma(msk_sb[:], msk_h.ap())
            for lo, hi in ((3 * QT, 5 * QT), (5 * QT, 7 * QT), (7 * QT, S)):
                dma(xtf_sb[:, :, lo:hi], xtf_r[:, :, lo:hi])

            # ones columns of V_ext (V blocks overwrite cols 0..63 later)
            nc.vector.memset(vext8[:, :, E:E + 1], 1.0)
            nc.vector.memset(vext16[:, :, E:E + 1], 1.0)

            msk_tri = msk_sb[:, 0:256].rearrange("p (a b) -> p a b", a=2)
            msk768 = msk_sb[:, 256:256 + 768]

            def even_blocks(ap2d, s4):
                """[128, 512] strided view: even 128-col blocks
                {8s4, 8s4+2, 8s4+4, 8s4+6} of a [128, S] AP."""
                seg = ap2d[:, 1024 * s4:1024 * (s4 + 1)]
                return seg.rearrange("p (b two x) -> p b two x",
                                     two=2, x=PB)[:, :, 0, :]

            def even_blocks_half(ap2d, h8):
                """[128, 256] strided view: even 128-col blocks of a
                512-col segment (half of an even_blocks segment)."""
                seg = ap2d[:, QT * h8:QT * (h8 + 1)]
                return seg.rearrange("p (b two x) -> p b two x",
                                     two=2, x=PB)[:, :, 0, :]

            # Projection chains: specs is a list of ('q', t) | ('k', s4)
            # | ('v', j).  Chains are interleaved per weight chunk so
            # consecutive matmuls alternate PSUM banks (hides PE drain) and
            # short V matmuls ride inside long 512-col streams (their
            # weight loads hide under the 512-col matmuls).
            def chains(*specs):
                tiles = [pp.tile([PB, QT], F32, tag="proj",
                                 name=f"{kind}_ps")
                         for kind, idx in specs]
                for c in range(4):
                    for (kind, idx), ps in zip(specs, tiles):
                        if kind == 'q':
                            nc.tensor.matmul(
                                ps[:], wq2_sb[:, c, :],
                                xtf_sb[:, c, QT * idx:QT * (idx + 1)],
                                start=(c == 0), stop=(c == 3))
                        elif kind == 'k':
                            nc.tensor.matmul(
                                ps[:], wk2_sb[:, c, :],
                                even_blocks(xtf_sb[:, c, :], idx),
                                start=(c == 0), stop=(c == 3))
                        elif kind == 'kh':
                            nc.tensor.matmul(
                                ps[:, 0:256], wk2_sb[:, c, :],
                                even_blocks_half(xtf_sb[:, c, :], idx),
                                start=(c == 0), stop=(c == 3))
                        else:
                            nc.tensor.matmul(
                                ps[:, 0:E],
                                xtf_sb[:, c, 2 * PB * idx:2 * PB * idx + PB],
                                wv1_sb[:, c, :],
                                start=(c == 0), stop=(c == 3))
                for (kind, idx), ps in zip(specs, tiles):
                    if kind == 'q':
                        nc.vector.tensor_copy(
                            qt2[:, QT * idx:QT * (idx + 1)], ps[:])
                    elif kind == 'k':
                        nc.vector.tensor_copy(
                            kt2[:, QT * idx:QT * (idx + 1)], ps[:])
                    elif kind == 'kh':
                        nc.vector.tensor_copy(
                            kt2[:, 256 * idx:256 * (idx + 1)], ps[:, 0:256])
                    else:
                        nc.vector.tensor_copy(
                            vext8[:, idx, 0:E], ps[:, 0:E])
                        if idx < 2:
                            nc.vector.tensor_copy(
                                vext16[:, idx, 0:E], ps[:, 0:E])

            # ---- main loop over query tiles ----
            pend = []       # deferred PV groups (keeps PE off ACT's tail)
            for t in range(NQT):
                # V blocks (2t, 2t+1) are only read by the deferred PV of
                # tile t, flushed during t+1 -- project them one tile late,
                # riding inside that tile's long chains.
                if t == 0:
                    chains(('q', 0))
                    chains(('kh', 0))        # kt2 cols 0:256 from X cols 0:512
                elif t == 1:
                    chains(('q', 1), ('q', 2), ('v', 0))
                    chains(('kh', 1), ('v', 1))   # kt2 cols 256:512
                elif t == 7:
                    chains(('q', 7), ('v', 12), ('v', 13))
                    chains(('v', 14), ('v', 15))
                elif t % 2 == 1:
                    chains(('q', t), ('q', t + 1), ('v', 2 * t - 2))
                    chains(('v', 2 * t - 1))
                else:
                    chains(('k', t // 2), ('v', 2 * t - 2), ('v', 2 * t - 1))

                z_ps = zp.tile([E + 1, QT], F32, tag="z", name="z_ps")
                njb = 2 * t + 2
                groups = [list(range(g, min(g + GJ, njb)))
                          for g in range(0, njb, GJ)]
                for js in groups:
                    s_ps = sp.tile([PB, GJ * QT], F32, tag="s", name="s_ps")
                    for j in js:
                        sl = j - js[0]
                        half = slice(0, 64) if j % 2 == 0 else slice(64, 128)
                        if j == 2 * t + 1:
                            # diagonal-odd block: cols [0,256) fully masked,
                            # compute only the live half
                            nc.tensor.matmul(
                                s_ps[:, QT * sl:QT * sl + 256],
                                kt2[half, PB * j:PB * (j + 1)],
                                qt2[half, QT * t + 256:QT * (t + 1)],
                                start=True, stop=True)
                        else:
                            nc.tensor.matmul(
                                s_ps[:, QT * sl:QT * (sl + 1)],
                                kt2[half, PB * j:PB * (j + 1)],
                                qt2[half, QT * t:QT * (t + 1)],
                                start=True, stop=True)

                    # flush deferred PV matmuls (keep a few in flight;
                    # drain harder on the last tile to shorten the tail)
                    lim = 6 if t < 7 else 2
                    if len(pend) >= lim:
                        _flush_pv(nc, pend.pop(0))

                    diag = js[-1] == 2 * t + 1
                    w = QT * len(js)
                    if diag:
                        w -= 256     # diagonal-odd block is half width
                    if t == 0:
                        pt = ptp.tile([PB, GJ * QT], BF, tag="pt0",
                                      name="pt0")
                    else:
                        pt = ptp.tile([PB, GJ * QT], F8, tag="pt", name="pt")
                    bt_offdiag = (not diag) and (
                        (t == 6 and js[0] == 6) or
                        (t == 7 and js[0] in (6, 10)))
                    if (diag and t in BT_TILES) or bt_offdiag:
                        # DVE bit-trick exp (+ fused mask on diag groups)
                        it16 = itp.tile([PB, GJ * QT], I16, tag="it",
                                        name="it16")
                        nc.vector.tensor_scalar(
                            it16[:, 0:w], s_ps[:, 0:w], BT_A, BT_BIAS,
                            mybir.AluOpType.mult, mybir.AluOpType.add)
                        if diag:
                            nc.vector.scalar_tensor_tensor(
                                pt[:, 0:w], it16[:, 0:w].bitcast(BF), 1.0,
                                msk768[:, 0:w],
                                mybir.AluOpType.mult, mybir.AluOpType.mult)
                        else:
                            nc.vector.tensor_copy(
                                pt[:, 0:w], it16[:, 0:w].bitcast(BF))
                    else:
                        nc.scalar.activation(pt[:, 0:w], s_ps[:, 0:w],
                                             mybir.ActivationFunctionType.Exp,
                                             scale=float(SCALE))
                        if diag:
                            # partial-mask windows: first 128 cols of the
                            # even block and of the odd live half
                            pt_win = pt[:, 0:QT + PB].rearrange(
                                "p (a b) -> p a b", b=PB)[:, 0:5:4, :]
                            nc.vector.tensor_mul(pt_win, pt_win, msk_tri)
                    pend.append((z_ps, vext8, vext16, pt, js, t))

                # attach Z evacuation of this tile to the last deferred group
                pend[-1] = pend[-1] + (zt, zsbp)

            # tail: flush remaining deferred groups
            for p in pend:
                _flush_pv(nc, p)

    nc.compile()
    return nc


def _flush_pv(nc, pend):
    """Emit the deferred PV matmul group (and Z evacuation if attached)."""
    z_ps, vext8, vext16, pt, js, t = pend[:6]
    a = js[0]
    if t == 0:
        # bf16 path (precision-critical first queries)
        nc.tensor.matmul(z_ps[:], vext16[:, 0, :], pt[:, 0:QT],
                         start=True, stop=False)
        nc.tensor.matmul(z_ps[:, 256:QT], vext16[:, 1, :],
                         pt[:, QT:QT + 256], start=False, stop=True)
    elif js[-1] == 2 * t + 1:
        # diagonal group: plain fp8 matmul for the even block's first 256
        # queries, DoubleRow for the shared last 256 queries
        nc.tensor.matmul(z_ps[:, 0:256], vext8[:, a, 0:E + 1],
                         pt[:, 0:256], start=False, stop=False)
        nc.tensor.matmul(
            z_ps[:, 256:QT], vext8[:, a:a + 2, 0:E + 1],
            pt[:, 256:QT + 256].rearrange("p (k q) -> p k q", k=2),
            start=False, stop=True, perf_mode=DR)
    else:
        # off-diagonal pair: one DoubleRow matmul (virtual contraction 256)
        nc.tensor.matmul(
            z_ps[:], vext8[:, a:a + 2, 0:E + 1],
            pt[:].rearrange("p (k q) -> p k q", k=2),
            start=(a == 0), stop=False, perf_mode=DR)
    if len(pend) > 6:
        zt, zsbp = pend[6], pend[7]
        z_sb = zsbp.tile([E + 1, QT], F32, tag="zsb", name="z_sb")
        nc.vector.tensor_copy(z_sb[:], z_ps[:])
        nc.sync.dma_start(zt[:, QT * t:QT * (t + 1)], z_sb[:])


def _get_nc():
    if "nc" not in _CACHE:
        _CACHE["nc"] = _build()
    return _CACHE["nc"]


def _host_inputs(X, Wq, Wk, Wv):
    """Per-core input maps. Core 2b+c: batch b, key parity c; X^T rotated
    left by 128*c columns."""
    w2 = lambda w: np.concatenate([w, w], axis=1).astype(BF16)
    wq2, wk2 = w2(Wq), w2(Wk)
    wv1 = Wv.astype(BF16)
    # masks: [tri|tri] (2x128 partial windows) + 768-wide full diag mask
    i = np.arange(PB)[:, None]
    tri = (i <= np.arange(PB)[None, :]).astype(BF16)
    v512 = np.arange(QT)[None, :]
    even_m = (i <= v512).astype(BF16)            # [128, 512]
    v256 = np.arange(256)[None, :]
    odd_m = (i <= v256).astype(BF16)             # [128, 256]
    msk = np.concatenate([tri, tri, even_m, odd_m], axis=1)  # [128, 1024]

    in_maps = []
    for b in range(B):
        xt = np.ascontiguousarray(np.asarray(X[b]).T).astype(BF16)
        for c in (0, 1):
            xtc = xt if c == 0 else np.ascontiguousarray(
                np.roll(xt, -PB * c, axis=1))
            in_maps.append({
                "xtf": xtc,
                "wq2": wq2, "wk2": wk2, "wv1": wv1, "msk": msk,
            })
    return in_maps


def _combine(results):
    Z = np.empty((B, S, E), np.float32)
    for b in range(B):
        za = results[2 * b]["zt"].astype(np.float32)
        zb = np.roll(results[2 * b + 1]["zt"].astype(np.float32),
                     PB, axis=1)     # un-rotate core B's query columns
        # B's wrapped query block (global q < 128) is garbage; A covers it.
        zb[:, 0:PB] = 0.0
        num = za[:E] + zb[:E]
        den = za[E] + zb[E]
        Z[b] = (num / den[None, :]).T
    return Z


def kernel(X, Wq, Wk, Wv, _trace=False, _tmpdir=None):
    from concourse.bass_utils import run_bass_kernel_spmd
    nc = _get_nc()
    in_maps = _host_inputs(X, Wq, Wk, Wv)
    kw = {}
    if _tmpdir is not None:
        kw["tmpdir"] = _tmpdir
    res = run_bass_kernel_spmd(nc, in_maps, core_ids=list(range(NCORES)),
                               trace=_trace, **kw)
    _CACHE["last"] = res
    return _combine(res.results)
